# revision 7
# baseline (speedup 1.0000x reference)
"""MoE (top-2, 8 experts) SwiGLU kernel for 8 Trainium2 NeuronCores.

Strategy (expert-parallel + H-split pairing for load balance):
  - Host: router matmul + top-2 + softmax (tiny), build per-expert token
    permutation (token-major order, capacity-truncated exactly like the
    reference's jnp.nonzero(size=CAPACITY)).
  - Experts are paired big-with-small by token count; pair i is assigned
    to cores (2i, 2i+1).  Each core computes BOTH experts of its pair on
    HALF of the hidden dimension (core 2i: h[0:1024], core 2i+1:
    h[1024:2048]).  Per-core matmul columns = C_big + C_small ~ 2048,
    which balances the PE load to within ~1% of perfect regardless of
    the routing skew (vs. max_e C_e for plain expert-parallel).
  - Device (fused, per half-expert): hT = (W^T x^T) * silu(V^T x^T) in
    [H/2, C] layout, then partial y = Wo[hhalf]^T-contract @ hT -> [D, C]
    (bf16 out).  All matmuls bf16 with fp32 PSUM accumulation.
  - PE warm-up: a short burst of dummy matmuls on memset SBUF scratch
    runs while the first weight/x DMAs land, so the HAM clock gate
    reaches 2.4 GHz before real work begins and the PE never idles cold.
  - DMA cost matters as much as bandwidth here.  Every tensor crossing
    HBM<->SBUF is staged in DRAM in its exact SBUF layout so transfers
    are per-partition contiguous and cheap to issue, and only the two
    hardware-DGE queues are used (Sync for weights, Scalar for x/y —
    GpSimd's software-DGE ring costs ~7us to drain at exit).  x chunk
    and y store DMAs are interleaved with the compute stream so issue
    order tracks need order.
  - Host: sum the two half-H partial y's per expert, inverse-permutation
    gather + weighted combine of the K=2 expert outputs per token.
"""

import numpy as np
import ml_dtypes

import concourse.bass as bass  # noqa: F401  (bass types referenced via bacc/tile)
import concourse.mybir as mybir
import concourse.tile as tile
from concourse import bacc
from concourse.bass_utils import run_bass_kernel_spmd

B, T = 2, 2048
D_MODEL, D_HIDDEN = 1024, 2048
N_EXPERTS, TOP_K = 8, 2
N_TOKENS = B * T
CAPACITY = 2 * N_TOKENS * TOP_K // N_EXPERTS  # 2048
H_HALF = D_HIDDEN // 2

F32 = mybir.dt.float32
BF16 = mybir.dt.bfloat16
AF = mybir.ActivationFunctionType
BF = ml_dtypes.bfloat16

_KERNEL_CACHE: dict = {}


def _chunk_plan(S: int, lead: int = 256, body: int = 512):
    """Column chunks: a modest lead chunk (so the first matmul group only
    waits for a small x DMA), then near-equal 128-aligned body chunks."""
    if S <= lead:
        return [(0, S)]
    chunks = [(0, lead)]
    rem = S - lead
    n = max(1, -(-rem // body))
    base = -(-rem // n // 128) * 128
    c0 = lead
    while c0 < S:
        cols = min(base, S - c0)
        chunks.append((c0, cols))
        c0 += cols
    return chunks


def _build_pair_kernel(S1: int, S2: int, D: int = D_MODEL, Hh: int = H_HALF,
                       chunk: int = 512, n_warm: int = 12):
    """Fused SwiGLU over two half-H experts: for s in (1, 2):
         y_s[D, S_s] = ((x_s @ W_s) * silu(x_s @ V_s)) @ Wo_s   (partial in H)
    """
    assert D % 128 == 0 and Hh % 128 == 0
    DK, HB, NB = D // 128, Hh // 128, D // 128
    nc = bacc.Bacc(None, target_bir_lowering=False, debug=False)

    subs = []
    for si, S in enumerate((S1, S2)):
        chunks = _chunk_plan(S, body=chunk)
        xs = [nc.dram_tensor(f"x{si}_{i}", [128, DK, cols], BF16,
                             kind="ExternalInput")
              for i, (_, cols) in enumerate(chunks)]
        # V and W interleaved per-hb so one DMA delivers a full slab pair
        WVd = nc.dram_tensor(f"WV{si}", [HB, 128, 2, DK, 128], BF16,
                             kind="ExternalInput")
        Wo = nc.dram_tensor(f"Wo{si}", [128, HB, D], BF16,
                            kind="ExternalInput")
        ys = [nc.dram_tensor(f"y{si}_{i}", [128, NB, cols], BF16,
                             kind="ExternalOutput")
              for i, (_, cols) in enumerate(chunks)]
        subs.append((si, S, chunks, xs, WVd, Wo, ys))

    with tile.TileContext(nc) as tc:
        with (
            tc.tile_pool(name="wpool", bufs=1) as wpool,
            tc.tile_pool(name="hpool", bufs=2) as hpool,
            tc.tile_pool(name="spool", bufs=3) as spool,
            tc.tile_pool(name="ypool", bufs=1) as ypool,
            tc.tile_pool(name="pa", bufs=2, space="PSUM") as pa_pool,
            tc.tile_pool(name="pb", bufs=2, space="PSUM") as pb_pool,
            tc.tile_pool(name="py", bufs=2, space="PSUM") as py_pool,
            tc.tile_pool(name="pw", bufs=1, space="PSUM") as pw_pool,
        ):
            # ---- PE warm-up ----
            wx = wpool.tile([128, 384], BF16, tag="warmx", name="warmx")
            ww = wpool.tile([128, 128], BF16, tag="warmw", name="warmw")
            wp = pw_pool.tile([128, 384], F32, tag="warmp")
            nc.vector.memset(wx[:], 0.0)
            nc.vector.memset(ww[:], 0.0)
            for _ in range(n_warm):
                nc.tensor.matmul(wp[:], ww[:], wx[:], start=True, stop=True)

            # ---- SBUF tiles ----
            plans = []
            for si, S, chunks, xs, WVd, Wo, ys in subs:
                x_tiles = [wpool.tile([128, DK, cols], BF16,
                                      tag=f"x{si}_{i}", name=f"x{si}_{i}")
                           for i, (_, cols) in enumerate(chunks)]
                WV_tiles = [wpool.tile([128, 2, DK, 128], BF16,
                                       tag=f"WV{si}_{hb}", name=f"WV{si}_{hb}")
                            for hb in range(HB)]
                Wo_sb = wpool.tile([128, HB, D], BF16, tag=f"Wo{si}")
                plans.append((si, S, chunks, x_tiles, WV_tiles,
                              Wo_sb, xs, WVd, Wo, ys))

            # ---- head DMA issue, need order.  Weights on the Sync HWDGE
            # queue; the first two x chunks of sub 0 on the Scalar HWDGE
            # queue (ahead of all its compute work).  Later x chunks are
            # issued inside the compute stream below.
            p0 = plans[0]
            nc.scalar.dma_start(out=p0[3][0][:], in_=p0[6][0][:])
            if len(p0[2]) > 1:
                nc.scalar.dma_start(out=p0[3][1][:], in_=p0[6][1][:])
            for hb in range(HB):
                nc.sync.dma_start(out=p0[4][hb][:], in_=p0[7][hb])
            nc.sync.dma_start(out=p0[5][:, :, :D // 2], in_=p0[8][:, :, :D // 2])
            nc.sync.dma_start(out=p0[5][:, :, D // 2:], in_=p0[8][:, :, D // 2:])
            p1 = plans[1]
            for hb in range(HB):
                nc.sync.dma_start(out=p1[4][hb][:], in_=p1[7][hb])
            nc.sync.dma_start(out=p1[5][:, :, :D // 2], in_=p1[8][:, :, :D // 2])
            nc.sync.dma_start(out=p1[5][:, :, D // 2:], in_=p1[8][:, :, D // 2:])

            # remaining x chunks, in global need order, attached to the
            # scalar stream at staggered points of the compute below
            x_feed = [(sp, i) for sp in plans for i in range(len(sp[2]))]
            x_feed = x_feed[2:]  # first two already issued
            feed_iter = iter(x_feed)

            def _feed_x(k=1):
                for _ in range(k):
                    nxt = next(feed_iter, None)
                    if nxt is None:
                        return
                    sp, i = nxt
                    nc.scalar.dma_start(out=sp[3][i][:], in_=sp[6][i][:])

            # ---- compute ----
            for (si, S, chunks, x_tiles, WV_tiles,
                 Wo_sb, xs, WVd, Wo, ys) in plans:
                for i, (c0, cols) in enumerate(chunks):
                    x_sb = x_tiles[i]
                    hT = hpool.tile([128, HB, chunk], BF16, tag="hT")
                    for hb in range(HB):
                        pa = pa_pool.tile([128, chunk], F32, tag="pa")
                        pb = pb_pool.tile([128, chunk], F32, tag="pb")
                        for dk in range(DK):
                            nc.tensor.matmul(
                                pb[:, :cols], WV_tiles[hb][:, 0, dk],
                                x_sb[:, dk, :cols],
                                start=(dk == 0), stop=(dk == DK - 1),
                            )
                        for dk in range(DK):
                            nc.tensor.matmul(
                                pa[:, :cols], WV_tiles[hb][:, 1, dk],
                                x_sb[:, dk, :cols],
                                start=(dk == 0), stop=(dk == DK - 1),
                            )
                        sg = spool.tile([128, chunk], F32, tag="sg")
                        nc.scalar.activation(sg[:, :cols], pb[:, :cols],
                                             AF.Silu)
                        nc.vector.tensor_mul(hT[:, hb, :cols], pa[:, :cols],
                                             sg[:, :cols])
                    last = (i == len(chunks) - 1)
                    ysb = ypool.tile([128, NB, cols], BF16, tag=f"ysb{si}_{i}")
                    for nb in range(NB):
                        py = py_pool.tile([128, chunk], F32, tag="py")
                        for hb in range(HB):
                            nc.tensor.matmul(
                                py[:, :cols],
                                Wo_sb[:, hb, nb * 128:(nb + 1) * 128],
                                hT[:, hb, :cols],
                                start=(hb == 0), stop=(hb == HB - 1),
                            )
                        nc.scalar.activation(ysb[:, nb, :], py[:, :cols],
                                             AF.Copy)
                        if last and nb == NB // 2 - 1:
                            nc.scalar.dma_start(out=ys[i][:, :NB // 2],
                                                in_=ysb[:, :NB // 2])
                    if last:
                        nc.scalar.dma_start(out=ys[i][:, NB // 2:],
                                            in_=ysb[:, NB // 2:])
                    else:
                        nc.scalar.dma_start(out=ys[i][:], in_=ysb[:])
                    _feed_x(1)
    nc.compile()
    return nc


def _get_kernel(S1: int, S2: int):
    key = (S1, S2)
    nc = _KERNEL_CACHE.get(key)
    if nc is None:
        nc = _build_pair_kernel(S1, S2)
        _KERNEL_CACHE[key] = nc
    return nc


def _router_logits(x_flat: np.ndarray, router_w: np.ndarray,
                   router_b: np.ndarray) -> np.ndarray:
    # Prefer jax-on-CPU so near-tie top-k decisions match the reference's
    # fp32 rounding as closely as possible; fall back to numpy.
    try:
        import jax
        import jax.numpy as jnp
        cpu = jax.devices("cpu")[0]
        with jax.default_device(cpu):
            lg = jnp.asarray(x_flat) @ jnp.asarray(router_w).T + jnp.asarray(router_b)
            return np.asarray(jax.device_get(lg)).astype(np.float32, copy=False)
    except Exception:
        return (x_flat @ router_w.T + router_b).astype(np.float32)


def _pack_wv(w, v):  # two [D, Hh] -> [HB, 128, 2, DK, 128] interleaved slabs
    Dm, Hm = w.shape
    ws = w.astype(BF).reshape(Dm // 128, 128, Hm // 128, 128).transpose(2, 1, 0, 3)
    vs = v.astype(BF).reshape(Dm // 128, 128, Hm // 128, 128).transpose(2, 1, 0, 3)
    return np.ascontiguousarray(np.stack([vs, ws], axis=2))


def _pack_wo(mat):  # [Hh, D] -> [128, HB, D] (partition = h within block)
    Hm, Dm = mat.shape
    return np.ascontiguousarray(
        mat.astype(BF).reshape(Hm // 128, 128, Dm).transpose(1, 0, 2))


def kernel(x, router_w, router_b, W, V, W_out):
    Bq, Tq, D = x.shape
    N = Bq * Tq
    x_flat = np.ascontiguousarray(x, dtype=np.float32).reshape(N, D)

    # ---- routing (host) ----
    logits = _router_logits(x_flat, router_w, router_b)          # [N, E]
    order2 = np.argsort(-logits, axis=1, kind="stable")[:, :TOP_K]  # lax.top_k ties
    top_ids = order2.astype(np.int64)                            # [N, K]
    top_vals = np.take_along_axis(logits, top_ids, axis=1)
    mx = top_vals.max(axis=1, keepdims=True)
    ex = np.exp((top_vals - mx).astype(np.float32))
    probs = (ex / ex.sum(axis=1, keepdims=True)).astype(np.float32)

    # ---- permutation (token-major scan order, capacity truncation) ----
    flat_e = top_ids.ravel()                                     # [N*K]
    scan = np.argsort(flat_e, kind="stable")                     # grouped by expert
    counts = np.bincount(flat_e, minlength=N_EXPERTS)
    starts = np.zeros(N_EXPERTS + 1, dtype=np.int64)
    starts[1:] = np.cumsum(counts)

    tok_e, slot_e, C_e = [], [], []
    for e in range(N_EXPERTS):
        idxs = scan[starts[e]:starts[e + 1]][:CAPACITY]
        tok_e.append(idxs // TOP_K)
        slot_e.append(idxs % TOP_K)
        C_e.append(max(1, len(idxs)))

    # ---- pairing: big with small, pair i -> cores (2i, 2i+1) ----
    order = np.argsort(-np.asarray(C_e), kind="stable")
    pairs = [(int(order[i]), int(order[N_EXPERTS - 1 - i]))
             for i in range(N_EXPERTS // 2)]
    S1 = max(C_e[p[0]] for p in pairs)
    S2 = max(C_e[p[1]] for p in pairs)

    # ---- per-core device inputs ----
    x_pad = np.vstack([x_flat, np.zeros((1, D), np.float32)])
    probs_pad = np.vstack([probs, np.zeros((1, TOP_K), np.float32)])

    def _pack_x(e, S):  # gathered tokens -> per-chunk [128, DK, cols] bf16
        idx = np.full(S, N, dtype=np.int64)
        idx[:len(tok_e[e])] = tok_e[e]
        xg = x_pad[idx]                                          # [S, D]
        xt = xg.T.astype(BF).reshape(D // 128, 128, S).transpose(1, 0, 2)
        return [np.ascontiguousarray(xt[:, :, c0:c0 + cols])
                for c0, cols in _chunk_plan(S)]

    x_packs = {}
    for pi, (eb, es) in enumerate(pairs):
        x_packs[eb] = _pack_x(eb, S1)
        x_packs[es] = _pack_x(es, S2)

    in_maps = []
    for pi, (eb, es) in enumerate(pairs):
        for half in range(2):
            h0 = half * H_HALF
            m = {}
            for si, e, S in ((0, eb, S1), (1, es, S2)):
                for i, xc in enumerate(x_packs[e]):
                    m[f"x{si}_{i}"] = xc
                m[f"WV{si}"] = _pack_wv(W[e][:, h0:h0 + H_HALF],
                                        V[e][:, h0:h0 + H_HALF])
                m[f"Wo{si}"] = _pack_wo(W_out[e][h0:h0 + H_HALF, :])
            in_maps.append(m)

    # ---- run on 8 cores ----
    nc = _get_kernel(S1, S2)
    res = None
    for attempt in range(2):
        try:
            res = run_bass_kernel_spmd(nc, in_maps,
                                       core_ids=list(range(N_EXPERTS)))
            break
        except Exception as err:  # transient axon/device errors: retry once
            import sys
            print(f"kernel: device run attempt {attempt} failed: {err!r}",
                  file=sys.stderr)

    # y per expert: sum of the two half-H partials -> [C_e, D] fp32
    def _unpack_y(r, si, S):  # chunked [128, NB, cols] -> [D, S] fp32
        out = np.empty((D, S), np.float32)
        for i, (c0, cols) in enumerate(_chunk_plan(S)):
            yc = r[f"y{si}_{i}"].astype(np.float32)      # [128, NB, cols]
            out[:, c0:c0 + cols] = yc.transpose(1, 0, 2).reshape(D, cols)
        return out

    y_exp = [None] * N_EXPERTS
    if res is not None:
        for pi, (eb, es) in enumerate(pairs):
            r0, r1 = res.results[2 * pi], res.results[2 * pi + 1]
            for si, e, S in ((0, eb, S1), (1, es, S2)):
                ysum = _unpack_y(r0, si, S) + _unpack_y(r1, si, S)
                y_exp[e] = ysum[:, :C_e[e]].T                # [C_e, D]
    else:  # last resort so a flaky device doesn't turn into a crash
        import sys
        print("kernel: falling back to host compute", file=sys.stderr)
        for e in range(N_EXPERTS):
            idx = np.full(C_e[e], N, dtype=np.int64)
            idx[:len(tok_e[e])] = tok_e[e]
            xg = x_pad[idx]
            a = xg @ W[e]
            b = xg @ V[e]
            y_exp[e] = (a * (b / (1.0 + np.exp(-b)))) @ W_out[e]

    # ---- combine (host): out[n] = sum_k w_k * y[expert_k(n), pos_k(n)] ----
    offs = np.zeros(N_EXPERTS + 1, dtype=np.int64)
    for e in range(N_EXPERTS):
        offs[e + 1] = offs[e] + C_e[e]
    pos_of_pair = np.full(N * TOP_K, offs[-1], dtype=np.int64)
    blocks = []
    for e in range(N_EXPERTS):
        ne = len(tok_e[e])
        idxs = tok_e[e] * TOP_K + slot_e[e]
        pos_of_pair[idxs] = offs[e] + np.arange(ne)
        w_e = probs_pad[tok_e[e], slot_e[e]][:, None].astype(np.float32)
        yb = np.zeros((C_e[e], D), np.float32)
        yb[:ne] = y_exp[e][:ne] * w_e
        blocks.append(yb)
    y_all = np.vstack(blocks + [np.zeros((1, D), np.float32)])
    out_flat = y_all[pos_of_pair].reshape(N, TOP_K, D).sum(axis=1)
    return out_flat.reshape(Bq, Tq, D).astype(np.float32, copy=False)


# revision 8
# speedup vs baseline: 1.0411x; 1.0411x over previous
"""MoE (top-2, 8 experts) SwiGLU kernel for 8 Trainium2 NeuronCores.

Strategy (expert-parallel + H-split pairing for load balance):
  - Host: router matmul + top-2 + softmax (tiny), build per-expert token
    permutation (token-major order, capacity-truncated exactly like the
    reference's jnp.nonzero(size=CAPACITY)).
  - Experts are paired big-with-small by token count; pair i is assigned
    to cores (2i, 2i+1).  Each core computes BOTH experts of its pair on
    HALF of the hidden dimension (core 2i: h[0:1024], core 2i+1:
    h[1024:2048]).  Per-core matmul columns = C_big + C_small ~ 2048,
    which balances the PE load to within ~1% of perfect regardless of
    the routing skew (vs. max_e C_e for plain expert-parallel).
  - Device (fused, per half-expert): hT = (W^T x^T) * silu(V^T x^T) in
    [H/2, C] layout, then partial y = Wo[hhalf]^T-contract @ hT -> [D, C]
    (bf16 out).  All matmuls bf16 with fp32 PSUM accumulation.
  - PE warm-up: a short burst of dummy matmuls on memset SBUF scratch
    runs while the first weight/x DMAs land, so the HAM clock gate
    reaches 2.4 GHz before real work begins and the PE never idles cold.
  - DMA cost matters as much as bandwidth here.  Every tensor crossing
    HBM<->SBUF is staged in DRAM in its exact SBUF layout so transfers
    are per-partition contiguous and cheap to issue, and only the two
    hardware-DGE queues are used (Sync for weights, Scalar for x/y —
    GpSimd's software-DGE ring costs ~7us to drain at exit).  x chunk
    and y store DMAs are interleaved with the compute stream so issue
    order tracks need order.
  - Host: sum the two half-H partial y's per expert, inverse-permutation
    gather + weighted combine of the K=2 expert outputs per token.
"""

import numpy as np
import ml_dtypes

import concourse.bass as bass  # noqa: F401  (bass types referenced via bacc/tile)
import concourse.mybir as mybir
import concourse.tile as tile
from concourse import bacc
from concourse.bass_utils import run_bass_kernel_spmd

B, T = 2, 2048
D_MODEL, D_HIDDEN = 1024, 2048
N_EXPERTS, TOP_K = 8, 2
N_TOKENS = B * T
CAPACITY = 2 * N_TOKENS * TOP_K // N_EXPERTS  # 2048
H_HALF = D_HIDDEN // 2

F32 = mybir.dt.float32
BF16 = mybir.dt.bfloat16
AF = mybir.ActivationFunctionType
BF = ml_dtypes.bfloat16

_KERNEL_CACHE: dict = {}


def _chunk_plan(S: int, lead: int = 512, body: int = 512):
    """Column chunks: a big lead chunk (so phase A consumes the streaming
    WV slabs no faster than HBM can supply them during the cold start),
    then a near-even split of the remainder."""
    if S <= lead:
        return [(0, S)]
    chunks = [(0, lead)]
    rem = S - lead
    n = max(1, -(-rem // body))
    c0 = lead
    for j in range(n):
        cols = rem // n + (1 if j < rem % n else 0)
        chunks.append((c0, cols))
        c0 += cols
    return chunks


def _build_pair_kernel(S1: int, S2: int, D: int = D_MODEL, Hh: int = H_HALF,
                       chunk: int = 512, n_warm: int = 12):
    """Fused SwiGLU over two half-H experts: for s in (1, 2):
         y_s[D, S_s] = ((x_s @ W_s) * silu(x_s @ V_s)) @ Wo_s   (partial in H)
    """
    assert D % 128 == 0 and Hh % 128 == 0
    DK, HB, NB = D // 128, Hh // 128, D // 128
    nc = bacc.Bacc(None, target_bir_lowering=False, debug=False)

    subs = []
    for si, S in enumerate((S1, S2)):
        chunks = _chunk_plan(S, body=chunk)
        xs = [nc.dram_tensor(f"x{si}_{i}", [128, DK, cols], BF16,
                             kind="ExternalInput")
              for i, (_, cols) in enumerate(chunks)]
        # V and W interleaved per-hb so one DMA delivers a full slab pair
        WVd = nc.dram_tensor(f"WV{si}", [HB, 128, 2, DK, 128], BF16,
                             kind="ExternalInput")
        Wo = nc.dram_tensor(f"Wo{si}", [128, HB, D], BF16,
                            kind="ExternalInput")
        ys = [nc.dram_tensor(f"y{si}_{i}", [128, NB, cols], BF16,
                             kind="ExternalOutput")
              for i, (_, cols) in enumerate(chunks)]
        subs.append((si, S, chunks, xs, WVd, Wo, ys))

    with tile.TileContext(nc) as tc:
        with (
            tc.tile_pool(name="wpool", bufs=1) as wpool,
            tc.tile_pool(name="hpool", bufs=2) as hpool,
            tc.tile_pool(name="spool", bufs=3) as spool,
            tc.tile_pool(name="ypool", bufs=1) as ypool,
            tc.tile_pool(name="pa", bufs=2, space="PSUM") as pa_pool,
            tc.tile_pool(name="pb", bufs=2, space="PSUM") as pb_pool,
            tc.tile_pool(name="py", bufs=2, space="PSUM") as py_pool,
            tc.tile_pool(name="pw", bufs=1, space="PSUM") as pw_pool,
        ):
            # ---- PE warm-up ----
            wx = wpool.tile([128, 384], BF16, tag="warmx", name="warmx")
            ww = wpool.tile([128, 128], BF16, tag="warmw", name="warmw")
            wp = pw_pool.tile([128, 384], F32, tag="warmp")
            nc.vector.memset(wx[:], 0.0)
            nc.vector.memset(ww[:], 0.0)
            for _ in range(n_warm):
                nc.tensor.matmul(wp[:], ww[:], wx[:], start=True, stop=True)

            # ---- SBUF tiles ----
            plans = []
            for si, S, chunks, xs, WVd, Wo, ys in subs:
                x_tiles = [wpool.tile([128, DK, cols], BF16,
                                      tag=f"x{si}_{i}", name=f"x{si}_{i}")
                           for i, (_, cols) in enumerate(chunks)]
                WV_tiles = [wpool.tile([128, 2, DK, 128], BF16,
                                       tag=f"WV{si}_{hb}", name=f"WV{si}_{hb}")
                            for hb in range(HB)]
                Wo_sb = wpool.tile([128, HB, D], BF16, tag=f"Wo{si}")
                plans.append((si, S, chunks, x_tiles, WV_tiles,
                              Wo_sb, xs, WVd, Wo, ys))

            # ---- head DMA issue, need order.  Weights on the Sync HWDGE
            # queue; the first two x chunks of sub 0 on the Scalar HWDGE
            # queue (ahead of all its compute work).  Later x chunks are
            # issued inside the compute stream below.
            p0 = plans[0]
            nc.scalar.dma_start(out=p0[3][0][:], in_=p0[6][0][:])
            if len(p0[2]) > 1:
                nc.scalar.dma_start(out=p0[3][1][:], in_=p0[6][1][:])
            for hb in range(HB):
                nc.sync.dma_start(out=p0[4][hb][:], in_=p0[7][hb])
            nc.sync.dma_start(out=p0[5][:, :HB // 2], in_=p0[8][:, :HB // 2])
            nc.sync.dma_start(out=p0[5][:, HB // 2:], in_=p0[8][:, HB // 2:])
            p1 = plans[1]
            for hb in range(HB):
                nc.sync.dma_start(out=p1[4][hb][:], in_=p1[7][hb])
            nc.sync.dma_start(out=p1[5][:, :HB // 2], in_=p1[8][:, :HB // 2])
            nc.sync.dma_start(out=p1[5][:, HB // 2:], in_=p1[8][:, HB // 2:])

            # remaining x chunks, in global need order, attached to the
            # scalar stream at staggered points of the compute below
            x_feed = [(sp, i) for sp in plans for i in range(len(sp[2]))]
            x_feed = x_feed[2:]  # first two already issued
            feed_iter = iter(x_feed)

            def _feed_x(k=1):
                for _ in range(k):
                    nxt = next(feed_iter, None)
                    if nxt is None:
                        return
                    sp, i = nxt
                    nc.scalar.dma_start(out=sp[3][i][:], in_=sp[6][i][:])

            # ---- compute ----
            for (si, S, chunks, x_tiles, WV_tiles,
                 Wo_sb, xs, WVd, Wo, ys) in plans:
                for i, (c0, cols) in enumerate(chunks):
                    x_sb = x_tiles[i]
                    hT = hpool.tile([128, HB, chunk], BF16, tag="hT")
                    for hb in range(HB):
                        pa = pa_pool.tile([128, chunk], F32, tag="pa")
                        pb = pb_pool.tile([128, chunk], F32, tag="pb")
                        for dk in range(DK):
                            nc.tensor.matmul(
                                pb[:, :cols], WV_tiles[hb][:, 0, dk],
                                x_sb[:, dk, :cols],
                                start=(dk == 0), stop=(dk == DK - 1),
                            )
                        for dk in range(DK):
                            nc.tensor.matmul(
                                pa[:, :cols], WV_tiles[hb][:, 1, dk],
                                x_sb[:, dk, :cols],
                                start=(dk == 0), stop=(dk == DK - 1),
                            )
                        sg = spool.tile([128, chunk], F32, tag="sg")
                        nc.scalar.activation(sg[:, :cols], pb[:, :cols],
                                             AF.Silu)
                        nc.vector.tensor_mul(hT[:, hb, :cols], pa[:, :cols],
                                             sg[:, :cols])
                    last = (i == len(chunks) - 1)
                    ysb = ypool.tile([128, NB, cols], BF16, tag=f"ysb{si}_{i}")
                    for nb in range(NB):
                        py = py_pool.tile([128, chunk], F32, tag="py")
                        for hb in range(HB):
                            nc.tensor.matmul(
                                py[:, :cols],
                                Wo_sb[:, hb, nb * 128:(nb + 1) * 128],
                                hT[:, hb, :cols],
                                start=(hb == 0), stop=(hb == HB - 1),
                            )
                        nc.vector.tensor_copy(ysb[:, nb, :], py[:, :cols])
                        if last and nb == NB // 2 - 1:
                            nc.scalar.dma_start(out=ys[i][:, :NB // 2],
                                                in_=ysb[:, :NB // 2])
                    if last:
                        nc.scalar.dma_start(out=ys[i][:, NB // 2:],
                                            in_=ysb[:, NB // 2:])
                    else:
                        nc.scalar.dma_start(out=ys[i][:], in_=ysb[:])
                    _feed_x(1)
    nc.compile()
    return nc


def _get_kernel(S1: int, S2: int):
    key = (S1, S2)
    nc = _KERNEL_CACHE.get(key)
    if nc is None:
        nc = _build_pair_kernel(S1, S2)
        _KERNEL_CACHE[key] = nc
    return nc


def _router_logits(x_flat: np.ndarray, router_w: np.ndarray,
                   router_b: np.ndarray) -> np.ndarray:
    # Prefer jax-on-CPU so near-tie top-k decisions match the reference's
    # fp32 rounding as closely as possible; fall back to numpy.
    try:
        import jax
        import jax.numpy as jnp
        cpu = jax.devices("cpu")[0]
        with jax.default_device(cpu):
            lg = jnp.asarray(x_flat) @ jnp.asarray(router_w).T + jnp.asarray(router_b)
            return np.asarray(jax.device_get(lg)).astype(np.float32, copy=False)
    except Exception:
        return (x_flat @ router_w.T + router_b).astype(np.float32)


def _pack_wv(w, v):  # two [D, Hh] -> [HB, 128, 2, DK, 128] interleaved slabs
    Dm, Hm = w.shape
    ws = w.astype(BF).reshape(Dm // 128, 128, Hm // 128, 128).transpose(2, 1, 0, 3)
    vs = v.astype(BF).reshape(Dm // 128, 128, Hm // 128, 128).transpose(2, 1, 0, 3)
    return np.ascontiguousarray(np.stack([vs, ws], axis=2))


def _pack_wo(mat):  # [Hh, D] -> [128, HB, D] (partition = h within block)
    Hm, Dm = mat.shape
    return np.ascontiguousarray(
        mat.astype(BF).reshape(Hm // 128, 128, Dm).transpose(1, 0, 2))


def kernel(x, router_w, router_b, W, V, W_out):
    Bq, Tq, D = x.shape
    N = Bq * Tq
    x_flat = np.ascontiguousarray(x, dtype=np.float32).reshape(N, D)

    # ---- routing (host) ----
    logits = _router_logits(x_flat, router_w, router_b)          # [N, E]
    order2 = np.argsort(-logits, axis=1, kind="stable")[:, :TOP_K]  # lax.top_k ties
    top_ids = order2.astype(np.int64)                            # [N, K]
    top_vals = np.take_along_axis(logits, top_ids, axis=1)
    mx = top_vals.max(axis=1, keepdims=True)
    ex = np.exp((top_vals - mx).astype(np.float32))
    probs = (ex / ex.sum(axis=1, keepdims=True)).astype(np.float32)

    # ---- permutation (token-major scan order, capacity truncation) ----
    flat_e = top_ids.ravel()                                     # [N*K]
    scan = np.argsort(flat_e, kind="stable")                     # grouped by expert
    counts = np.bincount(flat_e, minlength=N_EXPERTS)
    starts = np.zeros(N_EXPERTS + 1, dtype=np.int64)
    starts[1:] = np.cumsum(counts)

    tok_e, slot_e, C_e = [], [], []
    for e in range(N_EXPERTS):
        idxs = scan[starts[e]:starts[e + 1]][:CAPACITY]
        tok_e.append(idxs // TOP_K)
        slot_e.append(idxs % TOP_K)
        C_e.append(max(1, len(idxs)))

    # ---- pairing: big with small, pair i -> cores (2i, 2i+1) ----
    order = np.argsort(-np.asarray(C_e), kind="stable")
    pairs = [(int(order[i]), int(order[N_EXPERTS - 1 - i]))
             for i in range(N_EXPERTS // 2)]
    S1 = max(C_e[p[0]] for p in pairs)
    S2 = max(C_e[p[1]] for p in pairs)

    # ---- per-core device inputs ----
    x_pad = np.vstack([x_flat, np.zeros((1, D), np.float32)])
    probs_pad = np.vstack([probs, np.zeros((1, TOP_K), np.float32)])

    def _pack_x(e, S):  # gathered tokens -> per-chunk [128, DK, cols] bf16
        idx = np.full(S, N, dtype=np.int64)
        idx[:len(tok_e[e])] = tok_e[e]
        xg = x_pad[idx]                                          # [S, D]
        xt = xg.T.astype(BF).reshape(D // 128, 128, S).transpose(1, 0, 2)
        return [np.ascontiguousarray(xt[:, :, c0:c0 + cols])
                for c0, cols in _chunk_plan(S)]

    x_packs = {}
    for pi, (eb, es) in enumerate(pairs):
        x_packs[eb] = _pack_x(eb, S1)
        x_packs[es] = _pack_x(es, S2)

    in_maps = []
    for pi, (eb, es) in enumerate(pairs):
        for half in range(2):
            h0 = half * H_HALF
            m = {}
            for si, e, S in ((0, eb, S1), (1, es, S2)):
                for i, xc in enumerate(x_packs[e]):
                    m[f"x{si}_{i}"] = xc
                m[f"WV{si}"] = _pack_wv(W[e][:, h0:h0 + H_HALF],
                                        V[e][:, h0:h0 + H_HALF])
                m[f"Wo{si}"] = _pack_wo(W_out[e][h0:h0 + H_HALF, :])
            in_maps.append(m)

    # ---- run on 8 cores ----
    nc = _get_kernel(S1, S2)
    res = None
    for attempt in range(2):
        try:
            res = run_bass_kernel_spmd(nc, in_maps,
                                       core_ids=list(range(N_EXPERTS)))
            break
        except Exception as err:  # transient axon/device errors: retry once
            import sys
            print(f"kernel: device run attempt {attempt} failed: {err!r}",
                  file=sys.stderr)

    # y per expert: sum of the two half-H partials -> [C_e, D] fp32
    def _unpack_y(r, si, S):  # chunked [128, NB, cols] -> [D, S] fp32
        out = np.empty((D, S), np.float32)
        for i, (c0, cols) in enumerate(_chunk_plan(S)):
            yc = r[f"y{si}_{i}"].astype(np.float32)      # [128, NB, cols]
            out[:, c0:c0 + cols] = yc.transpose(1, 0, 2).reshape(D, cols)
        return out

    y_exp = [None] * N_EXPERTS
    if res is not None:
        for pi, (eb, es) in enumerate(pairs):
            r0, r1 = res.results[2 * pi], res.results[2 * pi + 1]
            for si, e, S in ((0, eb, S1), (1, es, S2)):
                ysum = _unpack_y(r0, si, S) + _unpack_y(r1, si, S)
                y_exp[e] = ysum[:, :C_e[e]].T                # [C_e, D]
    else:  # last resort so a flaky device doesn't turn into a crash
        import sys
        print("kernel: falling back to host compute", file=sys.stderr)
        for e in range(N_EXPERTS):
            idx = np.full(C_e[e], N, dtype=np.int64)
            idx[:len(tok_e[e])] = tok_e[e]
            xg = x_pad[idx]
            a = xg @ W[e]
            b = xg @ V[e]
            y_exp[e] = (a * (b / (1.0 + np.exp(-b)))) @ W_out[e]

    # ---- combine (host): out[n] = sum_k w_k * y[expert_k(n), pos_k(n)] ----
    offs = np.zeros(N_EXPERTS + 1, dtype=np.int64)
    for e in range(N_EXPERTS):
        offs[e + 1] = offs[e] + C_e[e]
    pos_of_pair = np.full(N * TOP_K, offs[-1], dtype=np.int64)
    blocks = []
    for e in range(N_EXPERTS):
        ne = len(tok_e[e])
        idxs = tok_e[e] * TOP_K + slot_e[e]
        pos_of_pair[idxs] = offs[e] + np.arange(ne)
        w_e = probs_pad[tok_e[e], slot_e[e]][:, None].astype(np.float32)
        yb = np.zeros((C_e[e], D), np.float32)
        yb[:ne] = y_exp[e][:ne] * w_e
        blocks.append(yb)
    y_all = np.vstack(blocks + [np.zeros((1, D), np.float32)])
    out_flat = y_all[pos_of_pair].reshape(N, TOP_K, D).sum(axis=1)
    return out_flat.reshape(Bq, Tq, D).astype(np.float32, copy=False)


# revision 9
# speedup vs baseline: 1.0495x; 1.0081x over previous
"""MoE (top-2, 8 experts) SwiGLU kernel for 8 Trainium2 NeuronCores.

Strategy (expert-parallel + H-split pairing for load balance):
  - Host: router matmul + top-2 + softmax (tiny), build per-expert token
    permutation (token-major order, capacity-truncated exactly like the
    reference's jnp.nonzero(size=CAPACITY)).
  - Experts are paired big-with-small by token count; pair i is assigned
    to cores (2i, 2i+1).  Each core computes BOTH experts of its pair on
    HALF of the hidden dimension (core 2i: h[0:1024], core 2i+1:
    h[1024:2048]).  Per-core matmul columns = C_big + C_small ~ 2048,
    which balances the PE load to within ~1% of perfect regardless of
    the routing skew (vs. max_e C_e for plain expert-parallel).
  - Device (fused, per half-expert): hT = (W^T x^T) * silu(V^T x^T) in
    [H/2, C] layout, then partial y = Wo[hhalf]^T-contract @ hT -> [D, C]
    (bf16 out).  All matmuls bf16 with fp32 PSUM accumulation.
  - PE warm-up: a short burst of dummy matmuls on memset SBUF scratch
    runs while the first weight/x DMAs land, so the HAM clock gate
    reaches 2.4 GHz before real work begins and the PE never idles cold.
  - DMA cost matters as much as bandwidth here.  Every tensor crossing
    HBM<->SBUF is staged in DRAM in its exact SBUF layout so transfers
    are per-partition contiguous and cheap to issue, and only the two
    hardware-DGE queues are used (Sync for weights, Scalar for x/y —
    GpSimd's software-DGE ring costs ~7us to drain at exit).  x chunk
    and y store DMAs are interleaved with the compute stream so issue
    order tracks need order.
  - Host: sum the two half-H partial y's per expert, inverse-permutation
    gather + weighted combine of the K=2 expert outputs per token.
"""

import numpy as np
import ml_dtypes

import concourse.bass as bass  # noqa: F401  (bass types referenced via bacc/tile)
import concourse.mybir as mybir
import concourse.tile as tile
from concourse import bacc
from concourse.bass_utils import run_bass_kernel_spmd

B, T = 2, 2048
D_MODEL, D_HIDDEN = 1024, 2048
N_EXPERTS, TOP_K = 8, 2
N_TOKENS = B * T
CAPACITY = 2 * N_TOKENS * TOP_K // N_EXPERTS  # 2048
H_HALF = D_HIDDEN // 2

F32 = mybir.dt.float32
BF16 = mybir.dt.bfloat16
AF = mybir.ActivationFunctionType
BF = ml_dtypes.bfloat16

_KERNEL_CACHE: dict = {}


def _chunk_plan(S: int, lead: int = 512, body: int = 512):
    """Column chunks: a big lead chunk (so phase A consumes the streaming
    WV slabs no faster than HBM can supply them during the cold start),
    then a near-even split of the remainder."""
    if S <= lead:
        return [(0, S)]
    chunks = [(0, lead)]
    rem = S - lead
    n = max(1, -(-rem // body))
    c0 = lead
    for j in range(n):
        cols = rem // n + (1 if j < rem % n else 0)
        chunks.append((c0, cols))
        c0 += cols
    return chunks


def _build_pair_kernel(S1: int, S2: int, D: int = D_MODEL, Hh: int = H_HALF,
                       chunk: int = 512, n_warm: int = 22):
    """Fused SwiGLU over two half-H experts: for s in (1, 2):
         y_s[D, S_s] = ((x_s @ W_s) * silu(x_s @ V_s)) @ Wo_s   (partial in H)
    """
    assert D % 128 == 0 and Hh % 128 == 0
    DK, HB, NB = D // 128, Hh // 128, D // 128
    nc = bacc.Bacc(None, target_bir_lowering=False, debug=False)

    subs = []
    for si, S in enumerate((S1, S2)):
        chunks = _chunk_plan(S, body=chunk)
        xs = [nc.dram_tensor(f"x{si}_{i}", [128, DK, cols], BF16,
                             kind="ExternalInput")
              for i, (_, cols) in enumerate(chunks)]
        # V and W interleaved per-hb so one DMA delivers a full slab pair
        WVd = nc.dram_tensor(f"WV{si}", [HB, 128, 2, DK, 128], BF16,
                             kind="ExternalInput")
        Wo = nc.dram_tensor(f"Wo{si}", [128, HB, D], BF16,
                            kind="ExternalInput")
        ys = [nc.dram_tensor(f"y{si}_{i}", [128, NB, cols], BF16,
                             kind="ExternalOutput")
              for i, (_, cols) in enumerate(chunks)]
        subs.append((si, S, chunks, xs, WVd, Wo, ys))

    with tile.TileContext(nc) as tc:
        with (
            tc.tile_pool(name="wpool", bufs=1) as wpool,
            tc.tile_pool(name="hpool", bufs=2) as hpool,
            tc.tile_pool(name="spool", bufs=3) as spool,
            tc.tile_pool(name="ypool", bufs=1) as ypool,
            tc.tile_pool(name="pa", bufs=2, space="PSUM") as pa_pool,
            tc.tile_pool(name="pb", bufs=2, space="PSUM") as pb_pool,
            tc.tile_pool(name="py", bufs=2, space="PSUM") as py_pool,
            tc.tile_pool(name="pw", bufs=1, space="PSUM") as pw_pool,
        ):
            # ---- PE warm-up ----
            wx = wpool.tile([128, 384], BF16, tag="warmx", name="warmx")
            ww = wpool.tile([128, 128], BF16, tag="warmw", name="warmw")
            wp = pw_pool.tile([128, 384], F32, tag="warmp")
            nc.vector.memset(wx[:], 0.0)
            nc.vector.memset(ww[:], 0.0)
            for _ in range(n_warm):
                nc.tensor.matmul(wp[:], ww[:], wx[:], start=True, stop=True)

            # ---- SBUF tiles ----
            plans = []
            for si, S, chunks, xs, WVd, Wo, ys in subs:
                x_tiles = [wpool.tile([128, DK, cols], BF16,
                                      tag=f"x{si}_{i}", name=f"x{si}_{i}")
                           for i, (_, cols) in enumerate(chunks)]
                WV_tiles = [wpool.tile([128, 2, DK, 128], BF16,
                                       tag=f"WV{si}_{hb}", name=f"WV{si}_{hb}")
                            for hb in range(HB)]
                Wo_sb = wpool.tile([128, HB, D], BF16, tag=f"Wo{si}")
                plans.append((si, S, chunks, x_tiles, WV_tiles,
                              Wo_sb, xs, WVd, Wo, ys))

            # ---- head DMA issue, need order.  Weights on the Sync HWDGE
            # queue; the first two x chunks of sub 0 on the Scalar HWDGE
            # queue (ahead of all its compute work).  Later x chunks are
            # issued inside the compute stream below.
            p0 = plans[0]
            DKh = DK // 2
            nc.scalar.dma_start(out=p0[3][0][:, :DKh], in_=p0[6][0][:, :DKh])
            nc.scalar.dma_start(out=p0[3][0][:, DKh:], in_=p0[6][0][:, DKh:])
            if len(p0[2]) > 1:
                nc.scalar.dma_start(out=p0[3][1][:], in_=p0[6][1][:])
            for hb in range(HB):
                nc.sync.dma_start(out=p0[4][hb][:], in_=p0[7][hb])
            nc.sync.dma_start(out=p0[5][:, :HB // 2], in_=p0[8][:, :HB // 2])
            nc.sync.dma_start(out=p0[5][:, HB // 2:], in_=p0[8][:, HB // 2:])
            p1 = plans[1]
            for hb in range(HB):
                nc.sync.dma_start(out=p1[4][hb][:], in_=p1[7][hb])
            nc.sync.dma_start(out=p1[5][:, :HB // 2], in_=p1[8][:, :HB // 2])
            nc.sync.dma_start(out=p1[5][:, HB // 2:], in_=p1[8][:, HB // 2:])

            # remaining x chunks, in global need order, attached to the
            # scalar stream at staggered points of the compute below
            x_feed = [(sp, i) for sp in plans for i in range(len(sp[2]))]
            x_feed = x_feed[2:]  # first two already issued
            feed_iter = iter(x_feed)

            def _feed_x(k=1):
                for _ in range(k):
                    nxt = next(feed_iter, None)
                    if nxt is None:
                        return
                    sp, i = nxt
                    nc.scalar.dma_start(out=sp[3][i][:], in_=sp[6][i][:])

            # ---- compute ----
            for (si, S, chunks, x_tiles, WV_tiles,
                 Wo_sb, xs, WVd, Wo, ys) in plans:
                for i, (c0, cols) in enumerate(chunks):
                    x_sb = x_tiles[i]
                    hT = hpool.tile([128, HB, chunk], BF16, tag="hT")
                    for hb in range(HB):
                        pa = pa_pool.tile([128, chunk], F32, tag="pa")
                        pb = pb_pool.tile([128, chunk], F32, tag="pb")
                        for dk in range(DK):
                            nc.tensor.matmul(
                                pb[:, :cols], WV_tiles[hb][:, 0, dk],
                                x_sb[:, dk, :cols],
                                start=(dk == 0), stop=(dk == DK - 1),
                            )
                        for dk in range(DK):
                            nc.tensor.matmul(
                                pa[:, :cols], WV_tiles[hb][:, 1, dk],
                                x_sb[:, dk, :cols],
                                start=(dk == 0), stop=(dk == DK - 1),
                            )
                        sg = spool.tile([128, chunk], F32, tag="sg")
                        nc.scalar.activation(sg[:, :cols], pb[:, :cols],
                                             AF.Silu)
                        nc.vector.tensor_mul(hT[:, hb, :cols], pa[:, :cols],
                                             sg[:, :cols])
                    last = (i == len(chunks) - 1)
                    ysb = ypool.tile([128, NB, cols], BF16, tag=f"ysb{si}_{i}")
                    for nb in range(NB):
                        py = py_pool.tile([128, chunk], F32, tag="py")
                        for hb in range(HB):
                            nc.tensor.matmul(
                                py[:, :cols],
                                Wo_sb[:, hb, nb * 128:(nb + 1) * 128],
                                hT[:, hb, :cols],
                                start=(hb == 0), stop=(hb == HB - 1),
                            )
                        nc.vector.tensor_copy(ysb[:, nb, :], py[:, :cols])
                        if last and si == 1:
                            nc.scalar.dma_start(out=ys[i][:, nb],
                                                in_=ysb[:, nb])
                    if not (last and si == 1):
                        nc.scalar.dma_start(out=ys[i][:], in_=ysb[:])
                    _feed_x(1)
    nc.compile()
    return nc


def _get_kernel(S1: int, S2: int):
    key = (S1, S2)
    nc = _KERNEL_CACHE.get(key)
    if nc is None:
        nc = _build_pair_kernel(S1, S2)
        _KERNEL_CACHE[key] = nc
    return nc


def _router_logits(x_flat: np.ndarray, router_w: np.ndarray,
                   router_b: np.ndarray) -> np.ndarray:
    # Prefer jax-on-CPU so near-tie top-k decisions match the reference's
    # fp32 rounding as closely as possible; fall back to numpy.
    try:
        import jax
        import jax.numpy as jnp
        cpu = jax.devices("cpu")[0]
        with jax.default_device(cpu):
            lg = jnp.asarray(x_flat) @ jnp.asarray(router_w).T + jnp.asarray(router_b)
            return np.asarray(jax.device_get(lg)).astype(np.float32, copy=False)
    except Exception:
        return (x_flat @ router_w.T + router_b).astype(np.float32)


def _pack_wv(w, v):  # two [D, Hh] -> [HB, 128, 2, DK, 128] interleaved slabs
    Dm, Hm = w.shape
    ws = w.astype(BF).reshape(Dm // 128, 128, Hm // 128, 128).transpose(2, 1, 0, 3)
    vs = v.astype(BF).reshape(Dm // 128, 128, Hm // 128, 128).transpose(2, 1, 0, 3)
    return np.ascontiguousarray(np.stack([vs, ws], axis=2))


def _pack_wo(mat):  # [Hh, D] -> [128, HB, D] (partition = h within block)
    Hm, Dm = mat.shape
    return np.ascontiguousarray(
        mat.astype(BF).reshape(Hm // 128, 128, Dm).transpose(1, 0, 2))


def kernel(x, router_w, router_b, W, V, W_out):
    Bq, Tq, D = x.shape
    N = Bq * Tq
    x_flat = np.ascontiguousarray(x, dtype=np.float32).reshape(N, D)

    # ---- routing (host) ----
    logits = _router_logits(x_flat, router_w, router_b)          # [N, E]
    order2 = np.argsort(-logits, axis=1, kind="stable")[:, :TOP_K]  # lax.top_k ties
    top_ids = order2.astype(np.int64)                            # [N, K]
    top_vals = np.take_along_axis(logits, top_ids, axis=1)
    mx = top_vals.max(axis=1, keepdims=True)
    ex = np.exp((top_vals - mx).astype(np.float32))
    probs = (ex / ex.sum(axis=1, keepdims=True)).astype(np.float32)

    # ---- permutation (token-major scan order, capacity truncation) ----
    flat_e = top_ids.ravel()                                     # [N*K]
    scan = np.argsort(flat_e, kind="stable")                     # grouped by expert
    counts = np.bincount(flat_e, minlength=N_EXPERTS)
    starts = np.zeros(N_EXPERTS + 1, dtype=np.int64)
    starts[1:] = np.cumsum(counts)

    tok_e, slot_e, C_e = [], [], []
    for e in range(N_EXPERTS):
        idxs = scan[starts[e]:starts[e + 1]][:CAPACITY]
        tok_e.append(idxs // TOP_K)
        slot_e.append(idxs % TOP_K)
        C_e.append(max(1, len(idxs)))

    # ---- pairing: big with small, pair i -> cores (2i, 2i+1) ----
    order = np.argsort(-np.asarray(C_e), kind="stable")
    pairs = [(int(order[i]), int(order[N_EXPERTS - 1 - i]))
             for i in range(N_EXPERTS // 2)]
    S1 = max(C_e[p[0]] for p in pairs)
    S2 = max(C_e[p[1]] for p in pairs)

    # ---- per-core device inputs ----
    x_pad = np.vstack([x_flat, np.zeros((1, D), np.float32)])
    probs_pad = np.vstack([probs, np.zeros((1, TOP_K), np.float32)])

    def _pack_x(e, S):  # gathered tokens -> per-chunk [128, DK, cols] bf16
        idx = np.full(S, N, dtype=np.int64)
        idx[:len(tok_e[e])] = tok_e[e]
        xg = x_pad[idx]                                          # [S, D]
        xt = xg.T.astype(BF).reshape(D // 128, 128, S).transpose(1, 0, 2)
        return [np.ascontiguousarray(xt[:, :, c0:c0 + cols])
                for c0, cols in _chunk_plan(S)]

    x_packs = {}
    for pi, (eb, es) in enumerate(pairs):
        x_packs[eb] = _pack_x(eb, S1)
        x_packs[es] = _pack_x(es, S2)

    in_maps = []
    for pi, (eb, es) in enumerate(pairs):
        for half in range(2):
            h0 = half * H_HALF
            m = {}
            for si, e, S in ((0, eb, S1), (1, es, S2)):
                for i, xc in enumerate(x_packs[e]):
                    m[f"x{si}_{i}"] = xc
                m[f"WV{si}"] = _pack_wv(W[e][:, h0:h0 + H_HALF],
                                        V[e][:, h0:h0 + H_HALF])
                m[f"Wo{si}"] = _pack_wo(W_out[e][h0:h0 + H_HALF, :])
            in_maps.append(m)

    # ---- run on 8 cores ----
    nc = _get_kernel(S1, S2)
    res = None
    for attempt in range(2):
        try:
            res = run_bass_kernel_spmd(nc, in_maps,
                                       core_ids=list(range(N_EXPERTS)))
            break
        except Exception as err:  # transient axon/device errors: retry once
            import sys
            print(f"kernel: device run attempt {attempt} failed: {err!r}",
                  file=sys.stderr)

    # y per expert: sum of the two half-H partials -> [C_e, D] fp32
    def _unpack_y(r, si, S):  # chunked [128, NB, cols] -> [D, S] fp32
        out = np.empty((D, S), np.float32)
        for i, (c0, cols) in enumerate(_chunk_plan(S)):
            yc = r[f"y{si}_{i}"].astype(np.float32)      # [128, NB, cols]
            out[:, c0:c0 + cols] = yc.transpose(1, 0, 2).reshape(D, cols)
        return out

    y_exp = [None] * N_EXPERTS
    if res is not None:
        for pi, (eb, es) in enumerate(pairs):
            r0, r1 = res.results[2 * pi], res.results[2 * pi + 1]
            for si, e, S in ((0, eb, S1), (1, es, S2)):
                ysum = _unpack_y(r0, si, S) + _unpack_y(r1, si, S)
                y_exp[e] = ysum[:, :C_e[e]].T                # [C_e, D]
    else:  # last resort so a flaky device doesn't turn into a crash
        import sys
        print("kernel: falling back to host compute", file=sys.stderr)
        for e in range(N_EXPERTS):
            idx = np.full(C_e[e], N, dtype=np.int64)
            idx[:len(tok_e[e])] = tok_e[e]
            xg = x_pad[idx]
            a = xg @ W[e]
            b = xg @ V[e]
            y_exp[e] = (a * (b / (1.0 + np.exp(-b)))) @ W_out[e]

    # ---- combine (host): out[n] = sum_k w_k * y[expert_k(n), pos_k(n)] ----
    offs = np.zeros(N_EXPERTS + 1, dtype=np.int64)
    for e in range(N_EXPERTS):
        offs[e + 1] = offs[e] + C_e[e]
    pos_of_pair = np.full(N * TOP_K, offs[-1], dtype=np.int64)
    blocks = []
    for e in range(N_EXPERTS):
        ne = len(tok_e[e])
        idxs = tok_e[e] * TOP_K + slot_e[e]
        pos_of_pair[idxs] = offs[e] + np.arange(ne)
        w_e = probs_pad[tok_e[e], slot_e[e]][:, None].astype(np.float32)
        yb = np.zeros((C_e[e], D), np.float32)
        yb[:ne] = y_exp[e][:ne] * w_e
        blocks.append(yb)
    y_all = np.vstack(blocks + [np.zeros((1, D), np.float32)])
    out_flat = y_all[pos_of_pair].reshape(N, TOP_K, D).sum(axis=1)
    return out_flat.reshape(Bq, Tq, D).astype(np.float32, copy=False)
